# revision 13
# baseline (speedup 1.0000x reference)
"""DeBERTa-style disentangled self-attention on 8 trn2 NeuronCores.

Sharding: core c handles batch b = c//4 and head-quad q = c%4 (heads 4q..4q+3):
data parallel over batch, tensor parallel over heads for the QKV/positional
projections and attention. The output dense is column-sharded (each core
computes dense columns 256q..256q+256 for its batch from the full attention
output, exchanged with one small per-head-pair AllGather), and the LayerNorm
row statistics are completed with a 16KB AllReduce.

Algebra: scores = q.kT + rel_q.kT + q.rel_kT = [q+rel_q; q] . [k; rel_k], so
the three score terms become one K=128 contraction. Softmax skips the
max-subtract (|scores*SCALE| stays small for these operand scales) and folds
the denominator as an extra all-ones column of V.
"""
import sys, types

sys.path.insert(0, '/opt/trn_rl_repo')


def _install_axon_hooks():
    if "antenv.axon_hooks" in sys.modules:
        return
    m = types.ModuleType("antenv.axon_hooks")
    state = {"hook": None}

    def set_axon_ntff_profile_hook(hook):
        state["hook"] = hook

    def get_axon_ntff_profile_hook():
        if state["hook"] is None:
            sys.path.insert(0, "/root/.axon_site/trn_agent_boot")
            import trn_boot
            state["hook"] = trn_boot._ntff_profile_via_ctypes("/opt/axon/libaxon_pjrt.so")
        return state["hook"]

    m.set_axon_ntff_profile_hook = set_axon_ntff_profile_hook
    m.get_axon_ntff_profile_hook = get_axon_ntff_profile_hook
    sys.modules["antenv.axon_hooks"] = m


_install_axon_hooks()

import numpy as np

import concourse.bass as bass
import concourse.bacc as bacc
import concourse.tile as tile
import concourse.mybir as mybir
from concourse.bass_utils import run_bass_kernel_spmd
from concourse.masks import make_identity

F32 = mybir.dt.float32
F32R = mybir.dt.float32r
F16 = mybir.dt.float16
AF = mybir.ActivationFunctionType
ALU = mybir.AluOpType
AX = mybir.AxisListType

B, N, H, D = 2, 2048, 16, 64
HID = H * D
NC = 8
HPC = 4            # heads per core
DHC = HPC * D      # 256-wide hid slice per core
EPS = 1e-7
SCALE = 1.0 / (3 * D) ** 0.5
GROUPS4 = [[0, 1, 2, 3], [4, 5, 6, 7]]

NCH = 4            # n-chunks for projections (512 each)
NCHW = N // NCH
JT = N // 128      # 16 j-tiles
IC = 2             # i-chunks for attention (1024 each)
ICW = N // IC
KT = HID // 128    # 8 contraction tiles
DA = D + 1         # V augmented with a ones column for the softmax denominator


def _build(flags, debug=False):
    use_qk_bias, use_bo, use_g, use_b = flags
    nc = bacc.Bacc("TRN2", target_bir_lowering=False, debug=False, num_devices=NC)

    def din(name, shape, dt):
        return nc.dram_tensor(name, shape, dt, kind="ExternalInput").ap()

    xT = din("xT", [HID, N], F32R)
    posT = din("posT", [HID, N], F32R)
    wq = din("wq", [HID, DHC], F32R)
    wk = din("wk", [HID, DHC], F32R)
    wv = din("wv", [HID, DHC], F32R)
    wpq = din("wpq", [HID, DHC], F32R)
    wpk = din("wpk", [HID, DHC], F32R)
    wo = din("wo", [HID, DHC], F16)
    xres = din("xres", [N, DHC], F32)
    bvp = din("bv", [DHC], F32)
    if use_qk_bias:
        qbias = din("qbias", [128, HPC], F32)   # rows 0:64 bq+bpq, 64:128 bq
        kbias = din("kbias", [128, HPC], F32)   # rows 0:64 bk, 64:128 bpk
    if use_bo:
        bop = din("bo", [DHC], F32)
    if use_g:
        gp = din("ln_g", [DHC], F32)
    if use_b:
        bp = din("ln_b", [DHC], F32)
    out = nc.dram_tensor("out", [N, DHC], F32, kind="ExternalOutput").ap()
    dbg = {}
    if debug:
        dbg["qcat0"] = nc.dram_tensor("dbg_qcat0", [128, N], F32, kind="ExternalOutput").ap()
        dbg["kcat0"] = nc.dram_tensor("dbg_kcat0", [128, N], F32, kind="ExternalOutput").ap()
        dbg["v"] = nc.dram_tensor("dbg_v", [128, JT, HPC, DA], F32, kind="ExternalOutput").ap()
        dbg["p0"] = nc.dram_tensor("dbg_p0", [128, JT, ICW], F32, kind="ExternalOutput").ap()
        dbg["avT0"] = nc.dram_tensor("dbg_avT0", [128, N], F32, kind="ExternalOutput").ap()
        dbg["stats"] = nc.dram_tensor("dbg_stats", [128, JT, 2], F32, kind="ExternalOutput").ap()
        dbg["dense0"] = nc.dram_tensor("dbg_dense0", [128, JT, DHC], F32, kind="ExternalOutput").ap()
        dbg["avfull"] = nc.dram_tensor("dbg_avfull", [128, KT, N], F16, kind="ExternalOutput").ap()

    kt_view = lambda t: t.rearrange("(kt p) m -> p kt m", p=128)

    def rep128(pool, src, tag, shape=None):
        """Replicate a 1-D DRAM vector across all 128 partitions."""
        ap = src if shape is None else src.rearrange(shape[0], **shape[1])
        t = pool.tile([128] + list(ap.shape), F32, tag=tag, name=tag)
        nc.sync.dma_start(
            out=t,
            in_=bass.AP(tensor=ap.tensor, offset=ap.offset,
                        ap=[[0, 128]] + [list(p) for p in ap.ap]))
        return t

    with tile.TileContext(nc) as tc:
        with (
            tc.tile_pool(name="const", bufs=1) as const,
            tc.tile_pool(name="qk", bufs=1) as qkp,
            tc.tile_pool(name="vb", bufs=1) as vbp,
            tc.tile_pool(name="avt", bufs=1) as avtp,
            tc.tile_pool(name="small", bufs=4) as small,
            tc.tile_pool(name="dram", bufs=1, space="DRAM") as dram,
        ):
            # ---- constants ----
            wo_t = const.tile([128, KT, DHC], F16, tag="wo")
            nc.sync.dma_start(out=wo_t, in_=kt_view(wo))
            ident = const.tile([128, 128], F16, tag="ident")
            make_identity(nc, ident)
            eps_t = const.tile([128, 1], F32, tag="eps")
            nc.vector.memset(eps_t, EPS)
            bv_rep = rep128(const, bvp, "bvrep", ("(h d) -> h d", dict(h=HPC)))
            if use_qk_bias:
                qb_t = const.tile([128, HPC], F32, tag="qb")
                kb_t = const.tile([128, HPC], F32, tag="kb")
                nc.sync.dma_start(out=qb_t, in_=qbias)
                nc.sync.dma_start(out=kb_t, in_=kbias)
            bo_rep = rep128(const, bop, "borep") if use_bo else None
            g_rep = rep128(const, gp, "grep") if use_g else None
            b_rep = rep128(const, bp, "brep") if use_b else None

            qcat = [qkp.tile([128, N], F32R, tag=f"qcat{h}", name=f"qcat{h}")
                    for h in range(HPC)]
            kcat = [qkp.tile([128, N], F32R, tag=f"kcat{h}", name=f"kcat{h}")
                    for h in range(HPC)]
            v_sb = vbp.tile([128, JT, HPC, DA], F16, tag="v")
            nc.vector.memset(v_sb, 1.0)
            avT = [avtp.tile([128, N], F16, tag=f"avT{pp}", name=f"avT{pp}")
                   for pp in range(2)]
            ag_in = [dram.tile([128, N], F16, tag=f"agin{pp}", name=f"agin{pp}")
                     for pp in range(2)]
            ag_out = [dram.tile([4, 128, N], F16, tag=f"agout{pp}",
                                name=f"agout{pp}") for pp in range(2)]

            # ---- phase 1: projections ----
            with (
                nc.named_scope("proj"),
                tc.tile_pool(name="wp", bufs=1) as wpl,
                tc.tile_pool(name="xtp", bufs=1 if debug else 2) as xtp,
                tc.tile_pool(name="ppsum", bufs=2, space="PSUM") as pps,
            ):
                w_t = {}
                for name, src in (("wq", wq), ("wk", wk), ("wv", wv),
                                  ("wpq", wpq), ("wpk", wpk)):
                    t = wpl.tile([128, KT, DHC], F32R, tag=name, name=name)
                    nc.sync.dma_start(out=t, in_=kt_view(src))
                    w_t[name] = t
                for nch in range(NCH):
                    ns = nch * NCHW
                    cs = slice(ns, ns + NCHW)
                    xt_c = xtp.tile([128, KT, NCHW], F32R, tag="xt")
                    nc.sync.dma_start(out=xt_c, in_=kt_view(xT)[:, :, cs])
                    pos_c = xtp.tile([128, KT, NCHW], F32R, tag="pos")
                    nc.sync.dma_start(out=pos_c, in_=kt_view(posT)[:, :, cs])
                    for pr in range(2):
                        ms = pr * 128
                        pq = pps.tile([128, NCHW], F32, tag="pq")
                        pk = pps.tile([128, NCHW], F32, tag="pk")
                        prk = pps.tile([128, NCHW], F32, tag="prk")
                        for kt in range(KT):
                            nc.tensor.matmul(pq, w_t["wq"][:, kt, ms:ms + 128],
                                             xt_c[:, kt, :],
                                             start=(kt == 0), stop=False)
                            nc.tensor.matmul(pk, w_t["wk"][:, kt, ms:ms + 128],
                                             xt_c[:, kt, :],
                                             start=(kt == 0), stop=(kt == KT - 1))
                            nc.tensor.matmul(prk, w_t["wpk"][:, kt, ms:ms + 128],
                                             pos_c[:, kt, :],
                                             start=(kt == 0), stop=(kt == KT - 1))
                        # evict plain q, then accumulate rel_q on top of pq
                        for hi in range(2):
                            h = pr * 2 + hi
                            sl = slice(64 * hi, 64 * hi + 64)
                            nc.vector.tensor_copy(out=qcat[h][64:128, cs],
                                                  in_=pq[sl, :])
                        for kt in range(KT):
                            nc.tensor.matmul(pq, w_t["wpq"][:, kt, ms:ms + 128],
                                             pos_c[:, kt, :],
                                             start=False, stop=(kt == KT - 1),
                                             skip_group_check=True)
                        for hi in range(2):
                            h = pr * 2 + hi
                            sl = slice(64 * hi, 64 * hi + 64)
                            nc.vector.tensor_copy(out=qcat[h][0:64, cs],
                                                  in_=pq[sl, :])
                            nc.vector.tensor_copy(out=kcat[h][0:64, cs],
                                                  in_=pk[sl, :])
                            nc.vector.tensor_copy(out=kcat[h][64:128, cs],
                                                  in_=prk[sl, :])
                            if use_qk_bias:
                                for tt, bt in ((qcat, qb_t), (kcat, kb_t)):
                                    nc.vector.tensor_scalar_add(
                                        out=tt[h][:, cs], in0=tt[h][:, cs],
                                        scalar1=bt[:, h:h + 1])
                    for jb in range(NCH):
                        jg = nch * NCH + jb
                        pv = pps.tile([128, DHC], F32, tag="pv")
                        for kt in range(KT):
                            nc.tensor.matmul(pv, xt_c[:, kt, jb * 128:jb * 128 + 128],
                                             w_t["wv"][:, kt, :],
                                             start=(kt == 0), stop=(kt == KT - 1))
                        nc.vector.tensor_add(
                            out=v_sb[:, jg, :, 0:D],
                            in0=pv.rearrange("p (h d) -> p h d", h=HPC),
                            in1=bv_rep)

            if debug:
                for name, src in (("qcat0", qcat[0]), ("kcat0", kcat[0])):
                    sb = small.tile([128, NCHW], F32, tag="dbgcast", name=f"dc_{name}", bufs=1)
                    for nch2 in range(NCH):
                        c2 = slice(nch2 * NCHW, nch2 * NCHW + NCHW)
                        nc.vector.tensor_copy(out=sb, in_=src[:, c2])
                        nc.sync.dma_start(out=dbg[name][:, c2], in_=sb)
                vcast = small.tile([128, HPC, DA], F32, tag="dbgv", name="dbgv", bufs=1)
                for jt2 in range(JT):
                    nc.vector.tensor_copy(out=vcast, in_=v_sb[:, jt2, :, :])
                    nc.sync.dma_start(out=dbg["v"][:, jt2, :, :], in_=vcast)

            # ---- phases 2+3: attention, allgather, dense, layernorm ----
            with (
                tc.tile_pool(name="pb", bufs=1 if debug else 2) as pbp,
                tc.tile_pool(name="apsum", bufs=1, space="PSUM") as aps,
                tc.tile_pool(name="dn", bufs=1) as dnp,
                tc.tile_pool(name="dsc", bufs=2) as dscp,
                tc.tile_pool(name="dpsum", bufs=1, space="PSUM") as dps,
            ):
                with nc.named_scope("attn"):
                    for h in range(HPC):
                        for ic in range(IC):
                            isb = ic * ICW
                            p_sb = pbp.tile([128, JT, ICW], F16, tag="psb")
                            for jp in range(JT // 2):
                                sp = aps.tile([128, 2, 2, 512], F32, tag="sp")
                                for jl in range(2):
                                    jt = jp * 2 + jl
                                    for hf in range(2):
                                        nc.tensor.matmul(
                                            sp[:, jl, hf, :],
                                            kcat[h][:, jt * 128:jt * 128 + 128],
                                            qcat[h][:, isb + hf * 512:
                                                    isb + hf * 512 + 512],
                                            start=True, stop=True)
                                nc.scalar.activation(
                                    out=p_sb[:, jp * 2:jp * 2 + 2, :]
                                        .rearrange("p a b -> p (a b)"),
                                    in_=sp.rearrange("p a b c -> p (a b c)"),
                                    func=AF.Exp, scale=SCALE)
                            if debug and h == 0 and ic == 0:
                                pcast = small.tile([128, ICW], F32, tag="dbgp", name="dbgp", bufs=1)
                                for jt2 in range(JT):
                                    nc.vector.tensor_copy(out=pcast, in_=p_sb[:, jt2, :])
                                    nc.sync.dma_start(out=dbg["p0"][:, jt2, :], in_=pcast)
                            for ib in range(ICW // 128):
                                ap2 = aps.tile([128, DA], F32, tag="av", bufs=2)
                                for jt in range(JT):
                                    nc.tensor.matmul(
                                        ap2, p_sb[:, jt, ib * 128:ib * 128 + 128],
                                        v_sb[:, jt, h, :],
                                        start=(jt == 0), stop=(jt == JT - 1))
                                r_t = small.tile([128, 1], F32, tag="r")
                                nc.vector.reciprocal(out=r_t, in_=ap2[:, D:DA])
                                av_t = small.tile([128, D], F16, tag="avsb")
                                nc.vector.tensor_scalar_mul(out=av_t,
                                                            in0=ap2[:, 0:D],
                                                            scalar1=r_t)
                                tp = aps.tile([64, 128], F16, tag="tp")
                                nc.tensor.transpose(tp, av_t, ident)
                                gi = isb + ib * 128
                                nc.vector.tensor_copy(
                                    out=avT[h // 2][64 * (h % 2):64 * (h % 2) + 64,
                                                    gi:gi + 128],
                                    in_=tp)
                        if h % 2 == 1:
                            pp = h // 2
                            if debug and pp == 0:
                                acast = small.tile([128, NCHW], F32, tag="dbga", name="dbga", bufs=1)
                                for nch2 in range(NCH):
                                    c2 = slice(nch2 * NCHW, nch2 * NCHW + NCHW)
                                    nc.vector.tensor_copy(out=acast, in_=avT[0][:, c2])
                                    nc.sync.dma_start(out=dbg["avT0"][:, c2], in_=acast)
                            nc.sync.dma_start(out=ag_in[pp], in_=avT[pp])
                            nc.gpsimd.collective_compute(
                                "AllGather", ALU.bypass, replica_groups=GROUPS4,
                                ins=[ag_in[pp].opt()], outs=[ag_out[pp].opt()])

                with nc.named_scope("dense"):
                    avfull = dnp.tile([128, KT, N], F16, tag="avfull")
                    avfull4 = avfull.rearrange("p (s t) n -> p s t n", t=2)
                    dense0 = dnp.tile([128, JT, DHC], F32, tag="d0")
                    # even half: pair-0 head columns, available right after AG0 —
                    # the scheduler overlaps this with attention on heads 2/3
                    nc.sync.dma_start(out=avfull4[:, :, 0, :],
                                      in_=ag_out[0].rearrange("s p n -> p s n"))
                    xres_v = xres.rearrange("(ib p) c -> p ib c", p=128)
                    for ib in range(JT):
                        xr = dscp.tile([128, DHC], F32, tag="xr")
                        nc.sync.dma_start(out=xr, in_=xres_v[:, ib, :])
                        pd = dps.tile([128, DHC], F32, tag="pd")
                        for s in range(4):
                            nc.tensor.matmul(pd, avfull4[:, s, 0,
                                                         ib * 128:ib * 128 + 128],
                                             wo_t[:, 2 * s, :],
                                             start=(s == 0), stop=(s == 3))
                        nc.vector.tensor_add(out=dense0[:, ib, :], in0=pd, in1=xr)
                    # odd half + stats, accumulated in place into dense0
                    nc.sync.dma_start(out=avfull4[:, :, 1, :],
                                      in_=ag_out[1].rearrange("s p n -> p s n"))
                    dense_t = dense0
                    stats = dnp.tile([128, JT, 2], F32, tag="stats")
                    for ib in range(JT):
                        pd = dps.tile([128, DHC], F32, tag="pd")
                        for s in range(4):
                            nc.tensor.matmul(pd, avfull4[:, s, 1,
                                                         ib * 128:ib * 128 + 128],
                                             wo_t[:, 2 * s + 1, :],
                                             start=(s == 0), stop=(s == 3))
                        dt_i = dense_t[:, ib, :]
                        nc.vector.tensor_add(out=dt_i, in0=pd, in1=dt_i)
                        if use_bo:
                            nc.vector.tensor_add(out=dt_i, in0=dt_i, in1=bo_rep)
                        nc.vector.reduce_sum(stats[:, ib, 0:1], dt_i, axis=AX.X)
                        sq = dscp.tile([128, DHC], F32, tag="sq")
                        nc.scalar.activation(out=sq, in_=dt_i, func=AF.Square,
                                             accum_out=stats[:, ib, 1:2])
                    if debug:
                        nc.sync.dma_start(out=dbg["avfull"], in_=avfull)
                        nc.sync.dma_start(out=dbg["stats"], in_=stats)
                        nc.sync.dma_start(out=dbg["dense0"], in_=dense_t)
                    ar_in = dram.tile([N, 2], F32, tag="arin")
                    ar_out = dram.tile([N, 2], F32, tag="arout")
                    nc.sync.dma_start(
                        out=ar_in.rearrange("(ib p) s -> p ib s", p=128), in_=stats)
                    nc.gpsimd.collective_compute(
                        "AllReduce", ALU.add, replica_groups=GROUPS4,
                        ins=[ar_in.opt()], outs=[ar_out.opt()])
                    stats2 = dnp.tile([128, JT, 2], F32, tag="stats2")
                    nc.sync.dma_start(
                        out=stats2, in_=ar_out.rearrange("(ib p) s -> p ib s", p=128))
                    inv_hid = 1.0 / HID
                    for ib in range(JT):
                        m_t = small.tile([128, 1], F32, tag="m")
                        v_t = small.tile([128, 1], F32, tag="vv")
                        sq_t = small.tile([128, 1], F32, tag="sqm")
                        nc.vector.tensor_scalar_mul(out=m_t, in0=stats2[:, ib, 0:1],
                                                    scalar1=inv_hid)
                        nc.vector.tensor_mul(out=sq_t, in0=m_t, in1=m_t)
                        nc.vector.tensor_scalar_mul(out=v_t, in0=stats2[:, ib, 1:2],
                                                    scalar1=inv_hid)
                        nc.vector.tensor_sub(out=v_t, in0=v_t, in1=sq_t)
                        nc.scalar.activation(out=v_t, in_=v_t, func=AF.Sqrt,
                                             bias=eps_t)
                        nc.vector.reciprocal(out=v_t, in_=v_t)
                        o_t = dscp.tile([128, DHC], F32, tag="ot")
                        nc.vector.tensor_scalar(out=o_t, in0=dense_t[:, ib, :],
                                                scalar1=m_t, scalar2=v_t,
                                                op0=ALU.subtract, op1=ALU.mult)
                        if use_g:
                            nc.vector.tensor_mul(out=o_t, in0=o_t, in1=g_rep)
                        if use_b:
                            nc.vector.tensor_add(out=o_t, in0=o_t, in1=b_rep)
                        nc.sync.dma_start(
                            out=out.rearrange("(ib p) c -> p ib c", p=128)[:, ib, :],
                            in_=o_t)

    nc.compile()
    return nc


def _flags(inp):
    return (bool(np.any(inp["bq"]) or np.any(inp["bk"]) or np.any(inp["bpq"])
                 or np.any(inp["bpk"])),
            bool(np.any(inp["bo"])),
            bool(np.any(np.asarray(inp["ln_g"]) != 1.0)),
            bool(np.any(inp["ln_b"])))


def _make_in_maps(inp, flags):
    use_qk_bias, use_bo, use_g, use_b = flags
    x = np.asarray(inp["hidden_states"], np.float32)
    xT = [np.ascontiguousarray(x[b].T) for b in range(B)]
    posT = np.ascontiguousarray(np.asarray(inp["pos_emb"], np.float32).T)
    in_maps = []
    for c in range(NC):
        b, q = c // 4, c % 4
        hs = slice(DHC * q, DHC * q + DHC)
        im = {
            "xT": xT[b],
            "posT": posT,
            "wq": np.ascontiguousarray(np.asarray(inp["Wq"], np.float32)[:, hs]),
            "wk": np.ascontiguousarray(np.asarray(inp["Wk"], np.float32)[:, hs]),
            "wv": np.ascontiguousarray(np.asarray(inp["Wv"], np.float32)[:, hs]),
            "wpq": np.ascontiguousarray(np.asarray(inp["Wpq"], np.float32)[:, hs]),
            "wpk": np.ascontiguousarray(np.asarray(inp["Wpk"], np.float32)[:, hs]),
            "wo": np.ascontiguousarray(np.asarray(inp["Wo"], np.float32)[:, hs])
                    .astype(np.float16),
            "xres": np.ascontiguousarray(x[b][:, hs]),
            "bv": np.ascontiguousarray(np.asarray(inp["bv"], np.float32)[hs]),
        }
        if use_qk_bias:
            bq = np.asarray(inp["bq"], np.float32)
            bk = np.asarray(inp["bk"], np.float32)
            bpq = np.asarray(inp["bpq"], np.float32)
            bpk = np.asarray(inp["bpk"], np.float32)
            qb = np.zeros((128, HPC), np.float32)
            kb = np.zeros((128, HPC), np.float32)
            for hh in range(HPC):
                ds = slice(DHC * q + D * hh, DHC * q + D * hh + D)
                qb[0:64, hh] = bq[ds] + bpq[ds]
                qb[64:128, hh] = bq[ds]
                kb[0:64, hh] = bk[ds]
                kb[64:128, hh] = bpk[ds]
            im["qbias"], im["kbias"] = qb, kb
        if use_bo:
            im["bo"] = np.ascontiguousarray(np.asarray(inp["bo"], np.float32)[hs])
        if use_g:
            im["ln_g"] = np.ascontiguousarray(np.asarray(inp["ln_g"], np.float32)[hs])
        if use_b:
            im["ln_b"] = np.ascontiguousarray(np.asarray(inp["ln_b"], np.float32)[hs])
        in_maps.append(im)
    return in_maps


def _assemble(results):
    out = np.empty((B, N, HID), np.float32)
    for c in range(NC):
        b, q = c // 4, c % 4
        out[b, :, DHC * q:DHC * q + DHC] = results[c]["out"]
    return out


_STATE = {}


def kernel(hidden_states, Wq, bq, Wk, bk, Wv, bv, pos_emb, Wpq, bpq, Wpk, bpk,
           Wo, bo, ln_g, ln_b):
    inp = dict(hidden_states=hidden_states, Wq=Wq, bq=bq, Wk=Wk, bk=bk, Wv=Wv,
               bv=bv, pos_emb=pos_emb, Wpq=Wpq, bpq=bpq, Wpk=Wpk, bpk=bpk,
               Wo=Wo, bo=bo, ln_g=ln_g, ln_b=ln_b)
    flags = _flags(inp)
    if flags not in _STATE:
        _STATE[flags] = _build(flags)
    nc = _STATE[flags]
    res = run_bass_kernel_spmd(nc, _make_in_maps(inp, flags), list(range(NC)))
    return _assemble(res.results)


# revision 15
# speedup vs baseline: 1.1819x; 1.1819x over previous
"""DeBERTa-style disentangled self-attention on 8 trn2 NeuronCores.

Sharding: core c handles batch b = c//4 and head-quad q = c%4 (heads 4q..4q+3):
data parallel over batch, tensor parallel over heads for the QKV/positional
projections and attention. The output dense is column-sharded (each core
computes dense columns 256q..256q+256 for its batch from the full attention
output, exchanged with one small per-head-pair AllGather), and the LayerNorm
row statistics are completed with a 16KB AllReduce.

Algebra: scores = q.kT + rel_q.kT + q.rel_kT = [q+rel_q; q] . [k; rel_k], so
the three score terms become one K=128 contraction. Softmax skips the
max-subtract (|scores*SCALE| stays small for these operand scales) and folds
the denominator as an extra all-ones column of V.
"""
import sys, types

sys.path.insert(0, '/opt/trn_rl_repo')


def _install_axon_hooks():
    if "antenv.axon_hooks" in sys.modules:
        return
    m = types.ModuleType("antenv.axon_hooks")
    state = {"hook": None}

    def set_axon_ntff_profile_hook(hook):
        state["hook"] = hook

    def get_axon_ntff_profile_hook():
        if state["hook"] is None:
            sys.path.insert(0, "/root/.axon_site/trn_agent_boot")
            import trn_boot
            state["hook"] = trn_boot._ntff_profile_via_ctypes("/opt/axon/libaxon_pjrt.so")
        return state["hook"]

    m.set_axon_ntff_profile_hook = set_axon_ntff_profile_hook
    m.get_axon_ntff_profile_hook = get_axon_ntff_profile_hook
    sys.modules["antenv.axon_hooks"] = m


_install_axon_hooks()

import numpy as np

import concourse.bass as bass
import concourse.bacc as bacc
import concourse.tile as tile
import concourse.mybir as mybir
from concourse.bass_utils import run_bass_kernel_spmd
from concourse.masks import make_identity

F32 = mybir.dt.float32
F32R = mybir.dt.float32r
F16 = mybir.dt.float16
AF = mybir.ActivationFunctionType
ALU = mybir.AluOpType
AX = mybir.AxisListType

B, N, H, D = 2, 2048, 16, 64
HID = H * D
NC = 8
HPC = 4            # heads per core
DHC = HPC * D      # 256-wide hid slice per core
EPS = 1e-7
SCALE = 1.0 / (3 * D) ** 0.5
GROUPS4 = [[0, 1, 2, 3], [4, 5, 6, 7]]

NCH = 2            # n-chunks for projections (1024 each)
NCHW = N // NCH
JBPC = NCHW // 128  # j-blocks per chunk
JT = N // 128      # 16 j-tiles
IC = 2             # i-chunks for attention (1024 each)
ICW = N // IC
KT = HID // 128    # 8 contraction tiles
DA = D + 1         # V augmented with a ones column for the softmax denominator


def _build(flags, debug=False):
    use_qk_bias, use_bo, use_g, use_b = flags
    nc = bacc.Bacc("TRN2", target_bir_lowering=False, debug=False, num_devices=NC)

    def din(name, shape, dt):
        return nc.dram_tensor(name, shape, dt, kind="ExternalInput").ap()

    xT = din("xT", [HID, N], F32R)
    posT = din("posT", [HID, N], F32R)
    wq = din("wq", [HID, DHC], F32R)
    wk = din("wk", [HID, DHC], F32R)
    wv = din("wv", [HID, DHC], F32R)
    wpq = din("wpq", [HID, DHC], F32R)
    wpk = din("wpk", [HID, DHC], F32R)
    wo = din("wo", [HID, DHC], F16)
    xres = din("xres", [N, DHC], F32)
    bvp = din("bv", [DHC], F32)
    if use_qk_bias:
        qbias = din("qbias", [128, HPC], F32)   # rows 0:64 bq+bpq, 64:128 bq
        kbias = din("kbias", [128, HPC], F32)   # rows 0:64 bk, 64:128 bpk
    if use_bo:
        bop = din("bo", [DHC], F32)
    if use_g:
        gp = din("ln_g", [DHC], F32)
    if use_b:
        bp = din("ln_b", [DHC], F32)
    out = nc.dram_tensor("out", [N, DHC], F32, kind="ExternalOutput").ap()
    dbg = {}
    if debug:
        dbg["qcat0"] = nc.dram_tensor("dbg_qcat0", [128, N], F32, kind="ExternalOutput").ap()
        dbg["kcat0"] = nc.dram_tensor("dbg_kcat0", [128, N], F32, kind="ExternalOutput").ap()
        dbg["v"] = nc.dram_tensor("dbg_v", [128, JT, HPC, DA], F32, kind="ExternalOutput").ap()
        dbg["p0"] = nc.dram_tensor("dbg_p0", [128, JT, ICW], F32, kind="ExternalOutput").ap()
        dbg["avT0"] = nc.dram_tensor("dbg_avT0", [128, N], F32, kind="ExternalOutput").ap()
        dbg["stats"] = nc.dram_tensor("dbg_stats", [128, JT, 2], F32, kind="ExternalOutput").ap()
        dbg["dense0"] = nc.dram_tensor("dbg_dense0", [128, JT, DHC], F32, kind="ExternalOutput").ap()
        dbg["avfull"] = nc.dram_tensor("dbg_avfull", [128, KT, N], F16, kind="ExternalOutput").ap()

    kt_view = lambda t: t.rearrange("(kt p) m -> p kt m", p=128)

    def rep128(pool, src, tag, shape=None):
        """Replicate a 1-D DRAM vector across all 128 partitions."""
        ap = src if shape is None else src.rearrange(shape[0], **shape[1])
        t = pool.tile([128] + list(ap.shape), F32, tag=tag, name=tag)
        nc.sync.dma_start(
            out=t,
            in_=bass.AP(tensor=ap.tensor, offset=ap.offset,
                        ap=[[0, 128]] + [list(p) for p in ap.ap]))
        return t

    with tile.TileContext(nc) as tc:
        with (
            tc.tile_pool(name="const", bufs=1) as const,
            tc.tile_pool(name="qk", bufs=1) as qkp,
            tc.tile_pool(name="vb", bufs=1) as vbp,
            tc.tile_pool(name="avt", bufs=1) as avtp,
            tc.tile_pool(name="small", bufs=4) as small,
            tc.tile_pool(name="dram", bufs=1, space="DRAM") as dram,
        ):
            # ---- constants ----
            wo_t = const.tile([128, KT, DHC], F16, tag="wo")
            nc.sync.dma_start(out=wo_t, in_=kt_view(wo))
            ident = const.tile([128, 128], F16, tag="ident")
            make_identity(nc, ident)
            eps_t = const.tile([128, 1], F32, tag="eps")
            nc.vector.memset(eps_t, EPS)
            bv_rep = rep128(const, bvp, "bvrep", ("(h d) -> h d", dict(h=HPC)))
            if use_qk_bias:
                qb_t = const.tile([128, HPC], F32, tag="qb")
                kb_t = const.tile([128, HPC], F32, tag="kb")
                nc.sync.dma_start(out=qb_t, in_=qbias)
                nc.sync.dma_start(out=kb_t, in_=kbias)
            bo_rep = rep128(const, bop, "borep") if use_bo else None
            g_rep = rep128(const, gp, "grep") if use_g else None
            b_rep = rep128(const, bp, "brep") if use_b else None

            qcat = [qkp.tile([128, N], F32R, tag=f"qcat{h}", name=f"qcat{h}")
                    for h in range(HPC)]
            kcat = [qkp.tile([128, N], F32R, tag=f"kcat{h}", name=f"kcat{h}")
                    for h in range(HPC)]
            v_sb = vbp.tile([128, JT, HPC, DA], F16, tag="v")
            nc.vector.memset(v_sb, 1.0)
            avT = [avtp.tile([128, N], F16, tag=f"avT{pp}", name=f"avT{pp}")
                   for pp in range(2)]
            ag_in = [dram.tile([128, N], F16, tag=f"agin{pp}", name=f"agin{pp}")
                     for pp in range(2)]
            ag_out = [dram.tile([4, 128, N], F16, tag=f"agout{pp}",
                                name=f"agout{pp}") for pp in range(2)]

            # ---- phase 1: projections ----
            with (
                nc.named_scope("proj"),
                tc.tile_pool(name="wp", bufs=1) as wpl,
                tc.tile_pool(name="xtp", bufs=1) as xtp,
                tc.tile_pool(name="ppsum", bufs=1, space="PSUM") as pps,
            ):
                w_t = {}
                for name, src in (("wq", wq), ("wk", wk), ("wv", wv),
                                  ("wpq", wpq), ("wpk", wpk)):
                    t = wpl.tile([128, KT, DHC], F32R, tag=name, name=name)
                    for kt in range(KT):
                        nc.sync.dma_start(out=t[:, kt, :], in_=kt_view(src)[:, kt, :])
                    w_t[name] = t
                for nch in range(NCH):
                    ns = nch * NCHW
                    cs = slice(ns, ns + NCHW)
                    xt_c = xtp.tile([128, KT, NCHW], F32R, tag="xt")
                    pos_c = xtp.tile([128, KT, NCHW], F32R, tag="pos")
                    for kt in range(KT):
                        nc.sync.dma_start(out=xt_c[:, kt, :],
                                          in_=kt_view(xT)[:, kt, cs])
                        nc.sync.dma_start(out=pos_c[:, kt, :],
                                          in_=kt_view(posT)[:, kt, cs])
                    for pr in range(2):
                        ms = pr * 128
                        pq = pps.tile([128, NCHW], F32, tag="pq")
                        pk = pps.tile([128, NCHW], F32, tag="pk")
                        prk = pps.tile([128, NCHW], F32, tag="prk")
                        for kt in range(KT):
                            for hf in range(NCHW // 512):
                                sl5 = slice(hf * 512, hf * 512 + 512)
                                nc.tensor.matmul(pq[:, sl5],
                                                 w_t["wq"][:, kt, ms:ms + 128],
                                                 xt_c[:, kt, sl5],
                                                 start=(kt == 0), stop=False)
                        for kt in range(KT):
                            for hf in range(NCHW // 512):
                                sl5 = slice(hf * 512, hf * 512 + 512)
                                nc.tensor.matmul(pk[:, sl5],
                                                 w_t["wk"][:, kt, ms:ms + 128],
                                                 xt_c[:, kt, sl5],
                                                 start=(kt == 0),
                                                 stop=(kt == KT - 1))
                        for kt in range(KT):
                            for hf in range(NCHW // 512):
                                sl5 = slice(hf * 512, hf * 512 + 512)
                                nc.tensor.matmul(prk[:, sl5],
                                                 w_t["wpk"][:, kt, ms:ms + 128],
                                                 pos_c[:, kt, sl5],
                                                 start=(kt == 0),
                                                 stop=(kt == KT - 1))
                        # evict plain q, then accumulate rel_q on top of pq
                        for hi in range(2):
                            h = pr * 2 + hi
                            sl = slice(64 * hi, 64 * hi + 64)
                            nc.vector.tensor_copy(out=qcat[h][64:128, cs],
                                                  in_=pq[sl, :])
                        for kt in range(KT):
                            for hf in range(NCHW // 512):
                                sl5 = slice(hf * 512, hf * 512 + 512)
                                nc.tensor.matmul(pq[:, sl5],
                                                 w_t["wpq"][:, kt, ms:ms + 128],
                                                 pos_c[:, kt, sl5],
                                                 start=False, stop=(kt == KT - 1),
                                                 skip_group_check=True)
                        for hi in range(2):
                            h = pr * 2 + hi
                            sl = slice(64 * hi, 64 * hi + 64)
                            nc.vector.tensor_copy(out=qcat[h][0:64, cs],
                                                  in_=pq[sl, :])
                            nc.vector.tensor_copy(out=kcat[h][0:64, cs],
                                                  in_=pk[sl, :])
                            nc.vector.tensor_copy(out=kcat[h][64:128, cs],
                                                  in_=prk[sl, :])
                            if use_qk_bias:
                                for tt, bt in ((qcat, qb_t), (kcat, kb_t)):
                                    nc.vector.tensor_scalar_add(
                                        out=tt[h][:, cs], in0=tt[h][:, cs],
                                        scalar1=bt[:, h:h + 1])
                    for jb in range(JBPC):
                        jg = nch * JBPC + jb
                        pv = pps.tile([128, DHC], F32, tag="pv", bufs=2)
                        for kt in range(KT):
                            nc.tensor.matmul(pv, xt_c[:, kt, jb * 128:jb * 128 + 128],
                                             w_t["wv"][:, kt, :],
                                             start=(kt == 0), stop=(kt == KT - 1))
                        nc.vector.tensor_add(
                            out=v_sb[:, jg, :, 0:D],
                            in0=pv.rearrange("p (h d) -> p h d", h=HPC),
                            in1=bv_rep)

            # ---- phases 2+3: attention, allgather, dense, layernorm ----
            with (
                tc.tile_pool(name="pb", bufs=1 if debug else 2) as pbp,
                tc.tile_pool(name="apsum", bufs=1, space="PSUM") as aps,
                tc.tile_pool(name="dn", bufs=1) as dnp,
                tc.tile_pool(name="dsc", bufs=2) as dscp,
                tc.tile_pool(name="dpsum", bufs=1, space="PSUM") as dps,
            ):
                with nc.named_scope("attn"):
                    for h in range(HPC):
                        for ic in range(IC):
                            isb = ic * ICW
                            p_sb = pbp.tile([128, JT, ICW], F16, tag="psb")
                            for jt in range(JT):
                                sp = aps.tile([128, ICW], F32, tag="sp", bufs=2)
                                for hf in range(2):
                                    nc.tensor.matmul(
                                        sp[:, hf * 512:hf * 512 + 512],
                                        kcat[h][:, jt * 128:jt * 128 + 128],
                                        qcat[h][:, isb + hf * 512:
                                                isb + hf * 512 + 512],
                                        start=True, stop=True)
                                nc.scalar.activation(
                                    out=p_sb[:, jt, :], in_=sp,
                                    func=AF.Exp, scale=SCALE)
                            if debug and h == 0 and ic == 0:
                                pcast = small.tile([128, ICW], F32, tag="dbgp", name="dbgp", bufs=1)
                                for jt2 in range(JT):
                                    nc.vector.tensor_copy(out=pcast, in_=p_sb[:, jt2, :])
                                    nc.sync.dma_start(out=dbg["p0"][:, jt2, :], in_=pcast)
                            for ib in range(ICW // 128):
                                ap2 = aps.tile([128, DA], F32, tag="av", bufs=2)
                                for jt in range(JT):
                                    nc.tensor.matmul(
                                        ap2, p_sb[:, jt, ib * 128:ib * 128 + 128],
                                        v_sb[:, jt, h, :],
                                        start=(jt == 0), stop=(jt == JT - 1))
                                r_t = small.tile([128, 1], F32, tag="r")
                                nc.vector.reciprocal(out=r_t, in_=ap2[:, D:DA])
                                av_t = small.tile([128, D], F16, tag="avsb")
                                nc.vector.tensor_scalar_mul(out=av_t,
                                                            in0=ap2[:, 0:D],
                                                            scalar1=r_t)
                                tp = aps.tile([64, 128], F16, tag="tp", bufs=1)
                                nc.tensor.transpose(tp, av_t, ident)
                                gi = isb + ib * 128
                                nc.vector.tensor_copy(
                                    out=avT[h // 2][64 * (h % 2):64 * (h % 2) + 64,
                                                    gi:gi + 128],
                                    in_=tp)
                        if h % 2 == 1:
                            pp = h // 2
                            if debug and pp == 0:
                                acast = small.tile([128, NCHW], F32, tag="dbga", name="dbga", bufs=1)
                                for nch2 in range(NCH):
                                    c2 = slice(nch2 * NCHW, nch2 * NCHW + NCHW)
                                    nc.vector.tensor_copy(out=acast, in_=avT[0][:, c2])
                                    nc.sync.dma_start(out=dbg["avT0"][:, c2], in_=acast)
                            nc.sync.dma_start(out=ag_in[pp], in_=avT[pp])
                            nc.gpsimd.collective_compute(
                                "AllGather", ALU.bypass, replica_groups=GROUPS4,
                                ins=[ag_in[pp].opt()], outs=[ag_out[pp].opt()])

                with nc.named_scope("dense"):
                    avfull = dnp.tile([128, KT, N], F16, tag="avfull")
                    avfull4 = avfull.rearrange("p (s t) n -> p s t n", t=2)
                    dense0 = dnp.tile([128, JT, DHC], F32, tag="d0")
                    # even half: pair-0 head columns, available right after AG0 —
                    # the scheduler overlaps this with attention on heads 2/3
                    nc.sync.dma_start(out=avfull4[:, :, 0, :],
                                      in_=ag_out[0].rearrange("s p n -> p s n"))
                    xres_v = xres.rearrange("(ib p) c -> p ib c", p=128)
                    for ib in range(JT):
                        xr = dscp.tile([128, DHC], F32, tag="xr")
                        nc.sync.dma_start(out=xr, in_=xres_v[:, ib, :])
                        pd = dps.tile([128, DHC], F32, tag="pd", bufs=1)
                        for s in range(4):
                            nc.tensor.matmul(pd, avfull4[:, s, 0,
                                                         ib * 128:ib * 128 + 128],
                                             wo_t[:, 2 * s, :],
                                             start=(s == 0), stop=(s == 3))
                        nc.vector.tensor_add(out=dense0[:, ib, :], in0=pd, in1=xr)
                    # odd half + stats, accumulated in place into dense0
                    nc.sync.dma_start(out=avfull4[:, :, 1, :],
                                      in_=ag_out[1].rearrange("s p n -> p s n"))
                    dense_t = dense0
                    stats = dnp.tile([128, JT, 2], F32, tag="stats")
                    for ib in range(JT):
                        pd = dps.tile([128, DHC], F32, tag="pd", bufs=1)
                        for s in range(4):
                            nc.tensor.matmul(pd, avfull4[:, s, 1,
                                                         ib * 128:ib * 128 + 128],
                                             wo_t[:, 2 * s + 1, :],
                                             start=(s == 0), stop=(s == 3))
                        dt_i = dense_t[:, ib, :]
                        nc.vector.tensor_add(out=dt_i, in0=pd, in1=dt_i)
                        if use_bo:
                            nc.vector.tensor_add(out=dt_i, in0=dt_i, in1=bo_rep)
                        nc.vector.reduce_sum(stats[:, ib, 0:1], dt_i, axis=AX.X)
                        sq = dscp.tile([128, DHC], F32, tag="sq")
                        nc.scalar.activation(out=sq, in_=dt_i, func=AF.Square,
                                             accum_out=stats[:, ib, 1:2])
                    if debug:
                        nc.sync.dma_start(out=dbg["avfull"], in_=avfull)
                        nc.sync.dma_start(out=dbg["stats"], in_=stats)
                        nc.sync.dma_start(out=dbg["dense0"], in_=dense_t)
                    ar_in = dram.tile([N, 2], F32, tag="arin")
                    ar_out = dram.tile([N, 2], F32, tag="arout")
                    nc.sync.dma_start(
                        out=ar_in.rearrange("(ib p) s -> p ib s", p=128), in_=stats)
                    nc.gpsimd.collective_compute(
                        "AllReduce", ALU.add, replica_groups=GROUPS4,
                        ins=[ar_in.opt()], outs=[ar_out.opt()])
                    stats2 = dnp.tile([128, JT, 2], F32, tag="stats2")
                    nc.sync.dma_start(
                        out=stats2, in_=ar_out.rearrange("(ib p) s -> p ib s", p=128))
                    inv_hid = 1.0 / HID
                    for ib in range(JT):
                        m_t = small.tile([128, 1], F32, tag="m")
                        v_t = small.tile([128, 1], F32, tag="vv")
                        sq_t = small.tile([128, 1], F32, tag="sqm")
                        nc.vector.tensor_scalar_mul(out=m_t, in0=stats2[:, ib, 0:1],
                                                    scalar1=inv_hid)
                        nc.vector.tensor_mul(out=sq_t, in0=m_t, in1=m_t)
                        nc.vector.tensor_scalar_mul(out=v_t, in0=stats2[:, ib, 1:2],
                                                    scalar1=inv_hid)
                        nc.vector.tensor_sub(out=v_t, in0=v_t, in1=sq_t)
                        nc.scalar.activation(out=v_t, in_=v_t, func=AF.Sqrt,
                                             bias=eps_t)
                        nc.vector.reciprocal(out=v_t, in_=v_t)
                        o_t = dscp.tile([128, DHC], F32, tag="ot")
                        nc.vector.tensor_scalar(out=o_t, in0=dense_t[:, ib, :],
                                                scalar1=m_t, scalar2=v_t,
                                                op0=ALU.subtract, op1=ALU.mult)
                        if use_g:
                            nc.vector.tensor_mul(out=o_t, in0=o_t, in1=g_rep)
                        if use_b:
                            nc.vector.tensor_add(out=o_t, in0=o_t, in1=b_rep)
                        nc.sync.dma_start(
                            out=out.rearrange("(ib p) c -> p ib c", p=128)[:, ib, :],
                            in_=o_t)

    nc.compile()
    return nc


def _flags(inp):
    return (bool(np.any(inp["bq"]) or np.any(inp["bk"]) or np.any(inp["bpq"])
                 or np.any(inp["bpk"])),
            bool(np.any(inp["bo"])),
            bool(np.any(np.asarray(inp["ln_g"]) != 1.0)),
            bool(np.any(inp["ln_b"])))


def _make_in_maps(inp, flags):
    use_qk_bias, use_bo, use_g, use_b = flags
    x = np.asarray(inp["hidden_states"], np.float32)
    xT = [np.ascontiguousarray(x[b].T) for b in range(B)]
    posT = np.ascontiguousarray(np.asarray(inp["pos_emb"], np.float32).T)
    in_maps = []
    for c in range(NC):
        b, q = c // 4, c % 4
        hs = slice(DHC * q, DHC * q + DHC)
        im = {
            "xT": xT[b],
            "posT": posT,
            "wq": np.ascontiguousarray(np.asarray(inp["Wq"], np.float32)[:, hs]),
            "wk": np.ascontiguousarray(np.asarray(inp["Wk"], np.float32)[:, hs]),
            "wv": np.ascontiguousarray(np.asarray(inp["Wv"], np.float32)[:, hs]),
            "wpq": np.ascontiguousarray(np.asarray(inp["Wpq"], np.float32)[:, hs]),
            "wpk": np.ascontiguousarray(np.asarray(inp["Wpk"], np.float32)[:, hs]),
            "wo": np.ascontiguousarray(np.asarray(inp["Wo"], np.float32)[:, hs])
                    .astype(np.float16),
            "xres": np.ascontiguousarray(x[b][:, hs]),
            "bv": np.ascontiguousarray(np.asarray(inp["bv"], np.float32)[hs]),
        }
        if use_qk_bias:
            bq = np.asarray(inp["bq"], np.float32)
            bk = np.asarray(inp["bk"], np.float32)
            bpq = np.asarray(inp["bpq"], np.float32)
            bpk = np.asarray(inp["bpk"], np.float32)
            qb = np.zeros((128, HPC), np.float32)
            kb = np.zeros((128, HPC), np.float32)
            for hh in range(HPC):
                ds = slice(DHC * q + D * hh, DHC * q + D * hh + D)
                qb[0:64, hh] = bq[ds] + bpq[ds]
                qb[64:128, hh] = bq[ds]
                kb[0:64, hh] = bk[ds]
                kb[64:128, hh] = bpk[ds]
            im["qbias"], im["kbias"] = qb, kb
        if use_bo:
            im["bo"] = np.ascontiguousarray(np.asarray(inp["bo"], np.float32)[hs])
        if use_g:
            im["ln_g"] = np.ascontiguousarray(np.asarray(inp["ln_g"], np.float32)[hs])
        if use_b:
            im["ln_b"] = np.ascontiguousarray(np.asarray(inp["ln_b"], np.float32)[hs])
        in_maps.append(im)
    return in_maps


def _assemble(results):
    out = np.empty((B, N, HID), np.float32)
    for c in range(NC):
        b, q = c // 4, c % 4
        out[b, :, DHC * q:DHC * q + DHC] = results[c]["out"]
    return out


_STATE = {}


def kernel(hidden_states, Wq, bq, Wk, bk, Wv, bv, pos_emb, Wpq, bpq, Wpk, bpk,
           Wo, bo, ln_g, ln_b):
    inp = dict(hidden_states=hidden_states, Wq=Wq, bq=bq, Wk=Wk, bk=bk, Wv=Wv,
               bv=bv, pos_emb=pos_emb, Wpq=Wpq, bpq=bpq, Wpk=Wpk, bpk=bpk,
               Wo=Wo, bo=bo, ln_g=ln_g, ln_b=ln_b)
    flags = _flags(inp)
    if flags not in _STATE:
        _STATE[flags] = _build(flags)
    nc = _STATE[flags]
    res = run_bass_kernel_spmd(nc, _make_in_maps(inp, flags), list(range(NC)))
    return _assemble(res.results)


# revision 16
# speedup vs baseline: 1.2838x; 1.0862x over previous
"""DeBERTa-style disentangled self-attention on 8 trn2 NeuronCores.

Sharding: core c handles batch b = c//4 and head-quad q = c%4 (heads 4q..4q+3):
data parallel over batch, tensor parallel over heads for the QKV/positional
projections and attention. The output dense is column-sharded (each core
computes dense columns 256q..256q+256 for its batch from the full attention
output, exchanged with one small per-head-pair AllGather), and the LayerNorm
row statistics are completed with a 16KB AllReduce.

Algebra: scores = q.kT + rel_q.kT + q.rel_kT = [q+rel_q; q] . [k; rel_k], so
the three score terms become one K=128 contraction. Softmax skips the
max-subtract (|scores*SCALE| stays small for these operand scales) and folds
the denominator as an extra all-ones column of V.
"""
import sys, types

sys.path.insert(0, '/opt/trn_rl_repo')


def _install_axon_hooks():
    if "antenv.axon_hooks" in sys.modules:
        return
    m = types.ModuleType("antenv.axon_hooks")
    state = {"hook": None}

    def set_axon_ntff_profile_hook(hook):
        state["hook"] = hook

    def get_axon_ntff_profile_hook():
        if state["hook"] is None:
            sys.path.insert(0, "/root/.axon_site/trn_agent_boot")
            import trn_boot
            state["hook"] = trn_boot._ntff_profile_via_ctypes("/opt/axon/libaxon_pjrt.so")
        return state["hook"]

    m.set_axon_ntff_profile_hook = set_axon_ntff_profile_hook
    m.get_axon_ntff_profile_hook = get_axon_ntff_profile_hook
    sys.modules["antenv.axon_hooks"] = m


_install_axon_hooks()

import numpy as np

import concourse.bass as bass
import concourse.bacc as bacc
import concourse.tile as tile
import concourse.mybir as mybir
from concourse.bass_utils import run_bass_kernel_spmd
from concourse.masks import make_identity

F32 = mybir.dt.float32
F32R = mybir.dt.float32r
F16 = mybir.dt.float16
AF = mybir.ActivationFunctionType
ALU = mybir.AluOpType
AX = mybir.AxisListType

B, N, H, D = 2, 2048, 16, 64
HID = H * D
NC = 8
HPC = 4            # heads per core
DHC = HPC * D      # 256-wide hid slice per core
EPS = 1e-7
SCALE = 1.0 / (3 * D) ** 0.5
GROUPS4 = [[0, 1, 2, 3], [4, 5, 6, 7]]

NCH = 4            # n-chunks for projections (512 each)
NCHW = N // NCH
JBPC = NCHW // 128  # j-blocks per chunk
JT = N // 128      # 16 j-tiles
IC = 2             # i-chunks for attention (1024 each)
ICW = N // IC
KT = HID // 128    # 8 contraction tiles
DA = D + 1         # V augmented with a ones column for the softmax denominator


def _build(flags, debug=False):
    use_qk_bias, use_bo, use_g, use_b = flags
    nc = bacc.Bacc("TRN2", target_bir_lowering=False, debug=False, num_devices=NC)

    def din(name, shape, dt):
        return nc.dram_tensor(name, shape, dt, kind="ExternalInput").ap()

    xT = din("xT", [HID, N], F16)
    posT = din("posT", [HID, N], F16)
    wq = din("wq", [HID, DHC], F16)
    wk = din("wk", [HID, DHC], F16)
    wv = din("wv", [HID, DHC], F16)
    wpq = din("wpq", [HID, DHC], F16)
    wpk = din("wpk", [HID, DHC], F16)
    wo = din("wo", [HID, DHC], F16)
    xres = din("xres", [N, DHC], F32)
    bvp = din("bv", [DHC], F32)
    if use_qk_bias:
        qbias = din("qbias", [128, HPC], F32)   # rows 0:64 bq+bpq, 64:128 bq
        kbias = din("kbias", [128, HPC], F32)   # rows 0:64 bk, 64:128 bpk
    if use_bo:
        bop = din("bo", [DHC], F32)
    if use_g:
        gp = din("ln_g", [DHC], F32)
    if use_b:
        bp = din("ln_b", [DHC], F32)
    out = nc.dram_tensor("out", [N, DHC], F32, kind="ExternalOutput").ap()
    dbg = {}
    if debug:
        dbg["qcat0"] = nc.dram_tensor("dbg_qcat0", [128, N], F32, kind="ExternalOutput").ap()
        dbg["kcat0"] = nc.dram_tensor("dbg_kcat0", [128, N], F32, kind="ExternalOutput").ap()
        dbg["v"] = nc.dram_tensor("dbg_v", [128, JT, HPC, DA], F32, kind="ExternalOutput").ap()
        dbg["p0"] = nc.dram_tensor("dbg_p0", [128, JT, ICW], F32, kind="ExternalOutput").ap()
        dbg["avT0"] = nc.dram_tensor("dbg_avT0", [128, N], F32, kind="ExternalOutput").ap()
        dbg["stats"] = nc.dram_tensor("dbg_stats", [128, JT, 2], F32, kind="ExternalOutput").ap()
        dbg["dense0"] = nc.dram_tensor("dbg_dense0", [128, JT, DHC], F32, kind="ExternalOutput").ap()
        dbg["avfull"] = nc.dram_tensor("dbg_avfull", [128, KT, N], F16, kind="ExternalOutput").ap()

    kt_view = lambda t: t.rearrange("(kt p) m -> p kt m", p=128)

    def rep128(pool, src, tag, shape=None):
        """Replicate a 1-D DRAM vector across all 128 partitions."""
        ap = src if shape is None else src.rearrange(shape[0], **shape[1])
        t = pool.tile([128] + list(ap.shape), F32, tag=tag, name=tag)
        nc.sync.dma_start(
            out=t,
            in_=bass.AP(tensor=ap.tensor, offset=ap.offset,
                        ap=[[0, 128]] + [list(p) for p in ap.ap]))
        return t

    with tile.TileContext(nc) as tc:
        with (
            tc.tile_pool(name="const", bufs=1) as const,
            tc.tile_pool(name="qk", bufs=1) as qkp,
            tc.tile_pool(name="vb", bufs=1) as vbp,
            tc.tile_pool(name="avt", bufs=1) as avtp,
            tc.tile_pool(name="small", bufs=4) as small,
            tc.tile_pool(name="dram", bufs=1, space="DRAM") as dram,
        ):
            # ---- constants ----
            wo_t = const.tile([128, KT, DHC], F16, tag="wo")
            nc.sync.dma_start(out=wo_t, in_=kt_view(wo))
            ident = const.tile([128, 128], F16, tag="ident")
            make_identity(nc, ident)
            eps_t = const.tile([128, 1], F32, tag="eps")
            nc.vector.memset(eps_t, EPS)
            bv_rep = rep128(const, bvp, "bvrep", ("(h d) -> h d", dict(h=HPC)))
            if use_qk_bias:
                qb_t = const.tile([128, HPC], F32, tag="qb")
                kb_t = const.tile([128, HPC], F32, tag="kb")
                nc.sync.dma_start(out=qb_t, in_=qbias)
                nc.sync.dma_start(out=kb_t, in_=kbias)
            bo_rep = rep128(const, bop, "borep") if use_bo else None
            g_rep = rep128(const, gp, "grep") if use_g else None
            b_rep = rep128(const, bp, "brep") if use_b else None

            qcat = [qkp.tile([128, N], F16, tag=f"qcat{h}", name=f"qcat{h}")
                    for h in range(HPC)]
            kcat = [qkp.tile([128, N], F16, tag=f"kcat{h}", name=f"kcat{h}")
                    for h in range(HPC)]
            v_sb = vbp.tile([128, JT, HPC, DA], F16, tag="v")
            nc.vector.memset(v_sb, 1.0)
            avT = [avtp.tile([128, N], F16, tag=f"avT{pp}", name=f"avT{pp}")
                   for pp in range(2)]
            ag_in = [dram.tile([128, N], F16, tag=f"agin{pp}", name=f"agin{pp}")
                     for pp in range(2)]
            ag_out = [dram.tile([4, 128, N], F16, tag=f"agout{pp}",
                                name=f"agout{pp}") for pp in range(2)]

            # ---- phase 1: projections ----
            with (
                nc.named_scope("proj"),
                tc.tile_pool(name="wp", bufs=1) as wpl,
                tc.tile_pool(name="xtp", bufs=1) as xtp,
                tc.tile_pool(name="ppsum", bufs=1, space="PSUM") as pps,
            ):
                w_t = {}
                for name, src in (("wq", wq), ("wk", wk), ("wv", wv),
                                  ("wpq", wpq), ("wpk", wpk)):
                    t = wpl.tile([128, KT, DHC], F16, tag=name, name=name)
                    for kt in range(KT):
                        nc.sync.dma_start(out=t[:, kt, :], in_=kt_view(src)[:, kt, :])
                    w_t[name] = t
                for nch in range(NCH):
                    ns = nch * NCHW
                    cs = slice(ns, ns + NCHW)
                    xt_c = xtp.tile([128, KT, NCHW], F16, tag="xt")
                    pos_c = xtp.tile([128, KT, NCHW], F16, tag="pos")
                    for kt in range(KT):
                        nc.sync.dma_start(out=xt_c[:, kt, :],
                                          in_=kt_view(xT)[:, kt, cs])
                        nc.sync.dma_start(out=pos_c[:, kt, :],
                                          in_=kt_view(posT)[:, kt, cs])
                    for pr in range(2):
                        ms = pr * 128
                        pq = pps.tile([128, NCHW], F32, tag="pq", bufs=2)
                        pk = pps.tile([128, NCHW], F32, tag="pk", bufs=2)
                        prk = pps.tile([128, NCHW], F32, tag="prk", bufs=2)
                        for kt in range(KT):
                            for hf in range(NCHW // 512):
                                sl5 = slice(hf * 512, hf * 512 + 512)
                                nc.tensor.matmul(pq[:, sl5],
                                                 w_t["wq"][:, kt, ms:ms + 128],
                                                 xt_c[:, kt, sl5],
                                                 start=(kt == 0), stop=False)
                        for kt in range(KT):
                            for hf in range(NCHW // 512):
                                sl5 = slice(hf * 512, hf * 512 + 512)
                                nc.tensor.matmul(pk[:, sl5],
                                                 w_t["wk"][:, kt, ms:ms + 128],
                                                 xt_c[:, kt, sl5],
                                                 start=(kt == 0),
                                                 stop=(kt == KT - 1))
                        for kt in range(KT):
                            for hf in range(NCHW // 512):
                                sl5 = slice(hf * 512, hf * 512 + 512)
                                nc.tensor.matmul(prk[:, sl5],
                                                 w_t["wpk"][:, kt, ms:ms + 128],
                                                 pos_c[:, kt, sl5],
                                                 start=(kt == 0),
                                                 stop=(kt == KT - 1))
                        # evict plain q, then accumulate rel_q on top of pq
                        for hi in range(2):
                            h = pr * 2 + hi
                            sl = slice(64 * hi, 64 * hi + 64)
                            nc.vector.tensor_copy(out=qcat[h][64:128, cs],
                                                  in_=pq[sl, :])
                        for kt in range(KT):
                            for hf in range(NCHW // 512):
                                sl5 = slice(hf * 512, hf * 512 + 512)
                                nc.tensor.matmul(pq[:, sl5],
                                                 w_t["wpq"][:, kt, ms:ms + 128],
                                                 pos_c[:, kt, sl5],
                                                 start=False, stop=(kt == KT - 1),
                                                 skip_group_check=True)
                        for hi in range(2):
                            h = pr * 2 + hi
                            sl = slice(64 * hi, 64 * hi + 64)
                            nc.vector.tensor_copy(out=qcat[h][0:64, cs],
                                                  in_=pq[sl, :])
                            nc.vector.tensor_copy(out=kcat[h][0:64, cs],
                                                  in_=pk[sl, :])
                            nc.vector.tensor_copy(out=kcat[h][64:128, cs],
                                                  in_=prk[sl, :])
                            if use_qk_bias:
                                for tt, bt in ((qcat, qb_t), (kcat, kb_t)):
                                    nc.vector.tensor_scalar_add(
                                        out=tt[h][:, cs], in0=tt[h][:, cs],
                                        scalar1=bt[:, h:h + 1])
                    for jb in range(JBPC):
                        jg = nch * JBPC + jb
                        pv = pps.tile([128, DHC], F32, tag="pv", bufs=2)
                        for kt in range(KT):
                            nc.tensor.matmul(pv, xt_c[:, kt, jb * 128:jb * 128 + 128],
                                             w_t["wv"][:, kt, :],
                                             start=(kt == 0), stop=(kt == KT - 1))
                        nc.vector.tensor_add(
                            out=v_sb[:, jg, :, 0:D],
                            in0=pv.rearrange("p (h d) -> p h d", h=HPC),
                            in1=bv_rep)

            # ---- phases 2+3: attention, allgather, dense, layernorm ----
            with (
                tc.tile_pool(name="pb", bufs=1 if debug else 2) as pbp,
                tc.tile_pool(name="apsum", bufs=1, space="PSUM") as aps,
                tc.tile_pool(name="dn", bufs=1) as dnp,
                tc.tile_pool(name="dsc", bufs=2) as dscp,
                tc.tile_pool(name="dpsum", bufs=1, space="PSUM") as dps,
            ):
                with nc.named_scope("attn"):
                    for h in range(HPC):
                        for ic in range(IC):
                            isb = ic * ICW
                            p_sb = pbp.tile([128, JT, ICW], F16, tag="psb")
                            for jt in range(JT):
                                sp = aps.tile([128, ICW], F32, tag="sp", bufs=2)
                                for hf in range(2):
                                    nc.tensor.matmul(
                                        sp[:, hf * 512:hf * 512 + 512],
                                        kcat[h][:, jt * 128:jt * 128 + 128],
                                        qcat[h][:, isb + hf * 512:
                                                isb + hf * 512 + 512],
                                        start=True, stop=True)
                                nc.scalar.activation(
                                    out=p_sb[:, jt, :], in_=sp,
                                    func=AF.Exp, scale=SCALE)
                            if debug and h == 0 and ic == 0:
                                pcast = small.tile([128, ICW], F32, tag="dbgp", name="dbgp", bufs=1)
                                for jt2 in range(JT):
                                    nc.vector.tensor_copy(out=pcast, in_=p_sb[:, jt2, :])
                                    nc.sync.dma_start(out=dbg["p0"][:, jt2, :], in_=pcast)
                            for ib in range(ICW // 128):
                                ap2 = aps.tile([128, DA], F32, tag="av", bufs=2)
                                for jt in range(JT):
                                    nc.tensor.matmul(
                                        ap2, p_sb[:, jt, ib * 128:ib * 128 + 128],
                                        v_sb[:, jt, h, :],
                                        start=(jt == 0), stop=(jt == JT - 1))
                                r_t = small.tile([128, 1], F32, tag="r")
                                nc.vector.reciprocal(out=r_t, in_=ap2[:, D:DA])
                                av_t = small.tile([128, D], F16, tag="avsb")
                                nc.vector.tensor_scalar_mul(out=av_t,
                                                            in0=ap2[:, 0:D],
                                                            scalar1=r_t)
                                tp = aps.tile([64, 128], F16, tag="tp", bufs=1)
                                nc.tensor.transpose(tp, av_t, ident)
                                gi = isb + ib * 128
                                nc.vector.tensor_copy(
                                    out=avT[h // 2][64 * (h % 2):64 * (h % 2) + 64,
                                                    gi:gi + 128],
                                    in_=tp)
                        if h % 2 == 1:
                            pp = h // 2
                            if debug and pp == 0:
                                acast = small.tile([128, NCHW], F32, tag="dbga", name="dbga", bufs=1)
                                for nch2 in range(NCH):
                                    c2 = slice(nch2 * NCHW, nch2 * NCHW + NCHW)
                                    nc.vector.tensor_copy(out=acast, in_=avT[0][:, c2])
                                    nc.sync.dma_start(out=dbg["avT0"][:, c2], in_=acast)
                            nc.sync.dma_start(out=ag_in[pp], in_=avT[pp])
                            nc.gpsimd.collective_compute(
                                "AllGather", ALU.bypass, replica_groups=GROUPS4,
                                ins=[ag_in[pp].opt()], outs=[ag_out[pp].opt()])

                with nc.named_scope("dense"):
                    avfull = dnp.tile([128, KT, N], F16, tag="avfull")
                    avfull4 = avfull.rearrange("p (s t) n -> p s t n", t=2)
                    dense0 = dnp.tile([128, JT, DHC], F32, tag="d0")
                    # even half: pair-0 head columns, available right after AG0 —
                    # the scheduler overlaps this with attention on heads 2/3
                    nc.sync.dma_start(out=avfull4[:, :, 0, :],
                                      in_=ag_out[0].rearrange("s p n -> p s n"))
                    xres_v = xres.rearrange("(ib p) c -> p ib c", p=128)
                    for ib in range(JT):
                        xr = dscp.tile([128, DHC], F32, tag="xr")
                        nc.sync.dma_start(out=xr, in_=xres_v[:, ib, :])
                        pd = dps.tile([128, DHC], F32, tag="pd", bufs=1)
                        for s in range(4):
                            nc.tensor.matmul(pd, avfull4[:, s, 0,
                                                         ib * 128:ib * 128 + 128],
                                             wo_t[:, 2 * s, :],
                                             start=(s == 0), stop=(s == 3))
                        nc.vector.tensor_add(out=dense0[:, ib, :], in0=pd, in1=xr)
                    # odd half + stats, accumulated in place into dense0
                    nc.sync.dma_start(out=avfull4[:, :, 1, :],
                                      in_=ag_out[1].rearrange("s p n -> p s n"))
                    dense_t = dense0
                    stats = dnp.tile([128, JT, 2], F32, tag="stats")
                    for ib in range(JT):
                        pd = dps.tile([128, DHC], F32, tag="pd", bufs=1)
                        for s in range(4):
                            nc.tensor.matmul(pd, avfull4[:, s, 1,
                                                         ib * 128:ib * 128 + 128],
                                             wo_t[:, 2 * s + 1, :],
                                             start=(s == 0), stop=(s == 3))
                        dt_i = dense_t[:, ib, :]
                        nc.vector.tensor_add(out=dt_i, in0=pd, in1=dt_i)
                        if use_bo:
                            nc.vector.tensor_add(out=dt_i, in0=dt_i, in1=bo_rep)
                        nc.vector.reduce_sum(stats[:, ib, 0:1], dt_i, axis=AX.X)
                        sq = dscp.tile([128, DHC], F32, tag="sq")
                        nc.scalar.activation(out=sq, in_=dt_i, func=AF.Square,
                                             accum_out=stats[:, ib, 1:2])
                    if debug:
                        nc.sync.dma_start(out=dbg["avfull"], in_=avfull)
                        nc.sync.dma_start(out=dbg["stats"], in_=stats)
                        nc.sync.dma_start(out=dbg["dense0"], in_=dense_t)
                    ar_in = dram.tile([N, 2], F32, tag="arin")
                    ar_out = dram.tile([N, 2], F32, tag="arout")
                    nc.sync.dma_start(
                        out=ar_in.rearrange("(ib p) s -> p ib s", p=128), in_=stats)
                    nc.gpsimd.collective_compute(
                        "AllReduce", ALU.add, replica_groups=GROUPS4,
                        ins=[ar_in.opt()], outs=[ar_out.opt()])
                    stats2 = dnp.tile([128, JT, 2], F32, tag="stats2")
                    nc.sync.dma_start(
                        out=stats2, in_=ar_out.rearrange("(ib p) s -> p ib s", p=128))
                    inv_hid = 1.0 / HID
                    m_all = dnp.tile([128, JT], F32, tag="mall")
                    v_all = dnp.tile([128, JT], F32, tag="vall")
                    sq_all = dnp.tile([128, JT], F32, tag="sqall")
                    nc.vector.tensor_scalar_mul(out=m_all, in0=stats2[:, :, 0],
                                                scalar1=inv_hid)
                    nc.vector.tensor_mul(out=sq_all, in0=m_all, in1=m_all)
                    nc.vector.tensor_scalar_mul(out=v_all, in0=stats2[:, :, 1],
                                                scalar1=inv_hid)
                    nc.vector.tensor_sub(out=v_all, in0=v_all, in1=sq_all)
                    nc.scalar.activation(out=v_all, in_=v_all, func=AF.Sqrt,
                                         bias=eps_t)
                    nc.vector.reciprocal(out=v_all, in_=v_all)
                    for ib in range(JT):
                        o_t = dscp.tile([128, DHC], F32, tag="ot")
                        nc.vector.tensor_scalar(out=o_t, in0=dense_t[:, ib, :],
                                                scalar1=m_all[:, ib:ib + 1],
                                                scalar2=v_all[:, ib:ib + 1],
                                                op0=ALU.subtract, op1=ALU.mult)
                        if use_g:
                            nc.vector.tensor_mul(out=o_t, in0=o_t, in1=g_rep)
                        if use_b:
                            nc.vector.tensor_add(out=o_t, in0=o_t, in1=b_rep)
                        nc.sync.dma_start(
                            out=out.rearrange("(ib p) c -> p ib c", p=128)[:, ib, :],
                            in_=o_t)

    nc.compile()
    return nc


def _flags(inp):
    return (bool(np.any(inp["bq"]) or np.any(inp["bk"]) or np.any(inp["bpq"])
                 or np.any(inp["bpk"])),
            bool(np.any(inp["bo"])),
            bool(np.any(np.asarray(inp["ln_g"]) != 1.0)),
            bool(np.any(inp["ln_b"])))


def _make_in_maps(inp, flags):
    use_qk_bias, use_bo, use_g, use_b = flags
    x = np.asarray(inp["hidden_states"], np.float32)
    xT = [np.ascontiguousarray(x[b].T) for b in range(B)]
    posT = np.ascontiguousarray(np.asarray(inp["pos_emb"], np.float32).T)
    in_maps = []
    for c in range(NC):
        b, q = c // 4, c % 4
        hs = slice(DHC * q, DHC * q + DHC)
        im = {
            "xT": xT[b].astype(np.float16),
            "posT": posT.astype(np.float16),
            "wq": np.ascontiguousarray(np.asarray(inp["Wq"], np.float32)[:, hs]).astype(np.float16),
            "wk": np.ascontiguousarray(np.asarray(inp["Wk"], np.float32)[:, hs]).astype(np.float16),
            "wv": np.ascontiguousarray(np.asarray(inp["Wv"], np.float32)[:, hs]).astype(np.float16),
            "wpq": np.ascontiguousarray(np.asarray(inp["Wpq"], np.float32)[:, hs]).astype(np.float16),
            "wpk": np.ascontiguousarray(np.asarray(inp["Wpk"], np.float32)[:, hs]).astype(np.float16),
            "wo": np.ascontiguousarray(np.asarray(inp["Wo"], np.float32)[:, hs])
                    .astype(np.float16),
            "xres": np.ascontiguousarray(x[b][:, hs]),
            "bv": np.ascontiguousarray(np.asarray(inp["bv"], np.float32)[hs]),
        }
        if use_qk_bias:
            bq = np.asarray(inp["bq"], np.float32)
            bk = np.asarray(inp["bk"], np.float32)
            bpq = np.asarray(inp["bpq"], np.float32)
            bpk = np.asarray(inp["bpk"], np.float32)
            qb = np.zeros((128, HPC), np.float32)
            kb = np.zeros((128, HPC), np.float32)
            for hh in range(HPC):
                ds = slice(DHC * q + D * hh, DHC * q + D * hh + D)
                qb[0:64, hh] = bq[ds] + bpq[ds]
                qb[64:128, hh] = bq[ds]
                kb[0:64, hh] = bk[ds]
                kb[64:128, hh] = bpk[ds]
            im["qbias"], im["kbias"] = qb, kb
        if use_bo:
            im["bo"] = np.ascontiguousarray(np.asarray(inp["bo"], np.float32)[hs])
        if use_g:
            im["ln_g"] = np.ascontiguousarray(np.asarray(inp["ln_g"], np.float32)[hs])
        if use_b:
            im["ln_b"] = np.ascontiguousarray(np.asarray(inp["ln_b"], np.float32)[hs])
        in_maps.append(im)
    return in_maps


def _assemble(results):
    out = np.empty((B, N, HID), np.float32)
    for c in range(NC):
        b, q = c // 4, c % 4
        out[b, :, DHC * q:DHC * q + DHC] = results[c]["out"]
    return out


_STATE = {}


def kernel(hidden_states, Wq, bq, Wk, bk, Wv, bv, pos_emb, Wpq, bpq, Wpk, bpk,
           Wo, bo, ln_g, ln_b):
    inp = dict(hidden_states=hidden_states, Wq=Wq, bq=bq, Wk=Wk, bk=bk, Wv=Wv,
               bv=bv, pos_emb=pos_emb, Wpq=Wpq, bpq=bpq, Wpk=Wpk, bpk=bpk,
               Wo=Wo, bo=bo, ln_g=ln_g, ln_b=ln_b)
    flags = _flags(inp)
    if flags not in _STATE:
        _STATE[flags] = _build(flags)
    nc = _STATE[flags]
    res = run_bass_kernel_spmd(nc, _make_in_maps(inp, flags), list(range(NC)))
    return _assemble(res.results)


# revision 18
# speedup vs baseline: 1.2862x; 1.0019x over previous
"""DeBERTa-style disentangled self-attention on 8 trn2 NeuronCores.

Sharding: core c handles batch b = c//4 and head-quad q = c%4 (heads 4q..4q+3):
data parallel over batch, tensor parallel over heads for the QKV/positional
projections and attention. The output dense is column-sharded (each core
computes dense columns 256q..256q+256 for its batch from the full attention
output, exchanged with one small per-head-pair AllGather), and the LayerNorm
row statistics are completed with a 16KB AllReduce.

Algebra: scores = q.kT + rel_q.kT + q.rel_kT = [q+rel_q; q] . [k; rel_k], so
the three score terms become one K=128 contraction. Softmax skips the
max-subtract (|scores*SCALE| stays small for these operand scales) and folds
the denominator as an extra all-ones column of V.
"""
import sys, types

sys.path.insert(0, '/opt/trn_rl_repo')


def _install_axon_hooks():
    if "antenv.axon_hooks" in sys.modules:
        return
    m = types.ModuleType("antenv.axon_hooks")
    state = {"hook": None}

    def set_axon_ntff_profile_hook(hook):
        state["hook"] = hook

    def get_axon_ntff_profile_hook():
        if state["hook"] is None:
            sys.path.insert(0, "/root/.axon_site/trn_agent_boot")
            import trn_boot
            state["hook"] = trn_boot._ntff_profile_via_ctypes("/opt/axon/libaxon_pjrt.so")
        return state["hook"]

    m.set_axon_ntff_profile_hook = set_axon_ntff_profile_hook
    m.get_axon_ntff_profile_hook = get_axon_ntff_profile_hook
    sys.modules["antenv.axon_hooks"] = m


_install_axon_hooks()

import numpy as np

import concourse.bass as bass
import concourse.bacc as bacc
import concourse.tile as tile
import concourse.mybir as mybir
from concourse.bass_utils import run_bass_kernel_spmd
from concourse.masks import make_identity

F32 = mybir.dt.float32
F32R = mybir.dt.float32r
F16 = mybir.dt.float16
AF = mybir.ActivationFunctionType
ALU = mybir.AluOpType
AX = mybir.AxisListType

B, N, H, D = 2, 2048, 16, 64
HID = H * D
NC = 8
HPC = 4            # heads per core
DHC = HPC * D      # 256-wide hid slice per core
EPS = 1e-7
SCALE = 1.0 / (3 * D) ** 0.5
GROUPS4 = [[0, 1, 2, 3], [4, 5, 6, 7]]

NCH = 2            # n-chunks for projections (1024 each)
NCHW = N // NCH
JBPC = NCHW // 128  # j-blocks per chunk
JT = N // 128      # 16 j-tiles
IC = 2             # i-chunks for attention (1024 each)
ICW = N // IC
KT = HID // 128    # 8 contraction tiles
DA = D + 1         # V augmented with a ones column for the softmax denominator


def _build(flags, debug=False):
    use_qk_bias, use_bo, use_g, use_b = flags
    nc = bacc.Bacc("TRN2", target_bir_lowering=False, debug=False, num_devices=NC)

    def din(name, shape, dt):
        return nc.dram_tensor(name, shape, dt, kind="ExternalInput").ap()

    xT = din("xT", [HID, N], F16)
    posT = din("posT", [HID, N], F16)
    wq = din("wq", [HID, DHC], F16)
    wk = din("wk", [HID, DHC], F16)
    wv = din("wv", [HID, DHC], F16)
    wpq = din("wpq", [HID, DHC], F16)
    wpk = din("wpk", [HID, DHC], F16)
    wo = din("wo", [HID, DHC], F16)
    xres = din("xres", [N, DHC], F32)
    bvp = din("bv", [DHC], F32)
    if use_qk_bias:
        qbias = din("qbias", [128, HPC], F32)   # rows 0:64 bq+bpq, 64:128 bq
        kbias = din("kbias", [128, HPC], F32)   # rows 0:64 bk, 64:128 bpk
    if use_bo:
        bop = din("bo", [DHC], F32)
    if use_g:
        gp = din("ln_g", [DHC], F32)
    if use_b:
        bp = din("ln_b", [DHC], F32)
    out = nc.dram_tensor("out", [N, DHC], F32, kind="ExternalOutput").ap()
    dbg = {}
    if debug:
        dbg["qcat0"] = nc.dram_tensor("dbg_qcat0", [128, N], F32, kind="ExternalOutput").ap()
        dbg["kcat0"] = nc.dram_tensor("dbg_kcat0", [128, N], F32, kind="ExternalOutput").ap()
        dbg["v"] = nc.dram_tensor("dbg_v", [128, JT, HPC, DA], F32, kind="ExternalOutput").ap()
        dbg["p0"] = nc.dram_tensor("dbg_p0", [128, JT, ICW], F32, kind="ExternalOutput").ap()
        dbg["avT0"] = nc.dram_tensor("dbg_avT0", [128, N], F32, kind="ExternalOutput").ap()
        dbg["stats"] = nc.dram_tensor("dbg_stats", [128, JT, 2], F32, kind="ExternalOutput").ap()
        dbg["dense0"] = nc.dram_tensor("dbg_dense0", [128, JT, DHC], F32, kind="ExternalOutput").ap()
        dbg["avfull"] = nc.dram_tensor("dbg_avfull", [128, KT, N], F16, kind="ExternalOutput").ap()

    kt_view = lambda t: t.rearrange("(kt p) m -> p kt m", p=128)

    def rep128(pool, src, tag, shape=None):
        """Replicate a 1-D DRAM vector across all 128 partitions."""
        ap = src if shape is None else src.rearrange(shape[0], **shape[1])
        t = pool.tile([128] + list(ap.shape), F32, tag=tag, name=tag)
        nc.sync.dma_start(
            out=t,
            in_=bass.AP(tensor=ap.tensor, offset=ap.offset,
                        ap=[[0, 128]] + [list(p) for p in ap.ap]))
        return t

    with tile.TileContext(nc) as tc:
        with (
            tc.tile_pool(name="const", bufs=1) as const,
            tc.tile_pool(name="qk", bufs=1) as qkp,
            tc.tile_pool(name="vb", bufs=1) as vbp,
            tc.tile_pool(name="avt", bufs=1) as avtp,
            tc.tile_pool(name="small", bufs=4) as small,
            tc.tile_pool(name="dram", bufs=1, space="DRAM") as dram,
        ):
            # ---- constants (non-critical loads are issued after the proj
            # weight/activation DMAs so the first matmuls start sooner) ----

            qcat = [qkp.tile([128, N], F16, tag=f"qcat{h}", name=f"qcat{h}")
                    for h in range(HPC)]
            kcat = [qkp.tile([128, N], F16, tag=f"kcat{h}", name=f"kcat{h}")
                    for h in range(HPC)]
            v_sb = vbp.tile([128, JT, HPC, DA], F16, tag="v")
            nc.vector.memset(v_sb, 1.0)
            avT = [avtp.tile([128, N], F16, tag=f"avT{pp}", name=f"avT{pp}")
                   for pp in range(2)]
            ag_in = [dram.tile([128, N], F16, tag=f"agin{pp}", name=f"agin{pp}")
                     for pp in range(2)]
            ag_out = [dram.tile([4, 128, N], F16, tag=f"agout{pp}",
                                name=f"agout{pp}") for pp in range(2)]

            # ---- phase 1: projections ----
            with (
                nc.named_scope("proj"),
                tc.tile_pool(name="wp", bufs=1) as wpl,
                tc.tile_pool(name="xtp", bufs=1) as xtp,
                tc.tile_pool(name="ppsum", bufs=1, space="PSUM") as pps,
            ):
                w_t = {}
                for name, src in (("wq", wq), ("wk", wk), ("wv", wv),
                                  ("wpq", wpq), ("wpk", wpk)):
                    t = wpl.tile([128, KT, DHC], F16, tag=name, name=name)
                    for kt in range(KT):
                        nc.sync.dma_start(out=t[:, kt, :], in_=kt_view(src)[:, kt, :])
                    w_t[name] = t
                wo_t = const.tile([128, KT, DHC], F16, tag="wo")
                nc.sync.dma_start(out=wo_t, in_=kt_view(wo))
                ident = const.tile([128, 128], F16, tag="ident")
                make_identity(nc, ident)
                eps_t = const.tile([128, 1], F32, tag="eps")
                nc.vector.memset(eps_t, EPS)
                bv_rep = rep128(const, bvp, "bvrep", ("(h d) -> h d", dict(h=HPC)))
                if use_qk_bias:
                    qb_t = const.tile([128, HPC], F32, tag="qb")
                    kb_t = const.tile([128, HPC], F32, tag="kb")
                    nc.sync.dma_start(out=qb_t, in_=qbias)
                    nc.sync.dma_start(out=kb_t, in_=kbias)
                bo_rep = rep128(const, bop, "borep") if use_bo else None
                g_rep = rep128(const, gp, "grep") if use_g else None
                b_rep = rep128(const, bp, "brep") if use_b else None
                for nch in range(NCH):
                    ns = nch * NCHW
                    cs = slice(ns, ns + NCHW)
                    xt_c = xtp.tile([128, KT, NCHW], F16, tag="xt")
                    pos_c = xtp.tile([128, KT, NCHW], F16, tag="pos")
                    for kt in range(KT):
                        nc.sync.dma_start(out=xt_c[:, kt, :],
                                          in_=kt_view(xT)[:, kt, cs])
                        nc.sync.dma_start(out=pos_c[:, kt, :],
                                          in_=kt_view(posT)[:, kt, cs])
                    for pr in range(2):
                        ms = pr * 128
                        pq = pps.tile([128, NCHW], F32, tag="pq")
                        pk = pps.tile([128, NCHW], F32, tag="pk")
                        prk = pps.tile([128, NCHW], F32, tag="prk")

                        def chain(dst, wname, src_c, start, stop):
                            w = w_t[wname]
                            for kt in range(KT):
                                for hf in range(NCHW // 512):
                                    sl5 = slice(hf * 512, hf * 512 + 512)
                                    nc.tensor.matmul(
                                        dst[:, sl5], w[:, kt, ms:ms + 128],
                                        src_c[:, kt, sl5],
                                        start=(start and kt == 0),
                                        stop=(stop and kt == KT - 1),
                                        skip_group_check=not start)

                        chain(pq, "wq", xt_c, True, False)
                        chain(pk, "wk", xt_c, True, True)
                        chain(prk, "wpk", pos_c, True, True)
                        # evict plain q, then accumulate rel_q on top of pq
                        for hi in range(2):
                            h = pr * 2 + hi
                            sl = slice(64 * hi, 64 * hi + 64)
                            nc.vector.tensor_copy(out=qcat[h][64:128, cs],
                                                  in_=pq[sl, :])
                        chain(pq, "wpq", pos_c, False, True)
                        for hi in range(2):
                            h = pr * 2 + hi
                            sl = slice(64 * hi, 64 * hi + 64)
                            nc.vector.tensor_copy(out=qcat[h][0:64, cs],
                                                  in_=pq[sl, :])
                            nc.vector.tensor_copy(out=kcat[h][0:64, cs],
                                                  in_=pk[sl, :])
                            nc.vector.tensor_copy(out=kcat[h][64:128, cs],
                                                  in_=prk[sl, :])
                            if use_qk_bias:
                                for tt, bt in ((qcat, qb_t), (kcat, kb_t)):
                                    nc.vector.tensor_scalar_add(
                                        out=tt[h][:, cs], in0=tt[h][:, cs],
                                        scalar1=bt[:, h:h + 1])
                    for jb in range(JBPC):
                        jg = nch * JBPC + jb
                        pv = pps.tile([128, DHC], F32, tag="pv", bufs=2)
                        for kt in range(KT):
                            nc.tensor.matmul(pv, xt_c[:, kt, jb * 128:jb * 128 + 128],
                                             w_t["wv"][:, kt, :],
                                             start=(kt == 0), stop=(kt == KT - 1))
                        nc.vector.tensor_add(
                            out=v_sb[:, jg, :, 0:D],
                            in0=pv.rearrange("p (h d) -> p h d", h=HPC),
                            in1=bv_rep)

            # ---- phases 2+3: attention, allgather, dense, layernorm ----
            with (
                tc.tile_pool(name="pb", bufs=1 if debug else 3) as pbp,
                tc.tile_pool(name="apsum", bufs=1, space="PSUM") as aps,
                tc.tile_pool(name="dn", bufs=1) as dnp,
                tc.tile_pool(name="dsc", bufs=2) as dscp,
                tc.tile_pool(name="dpsum", bufs=1, space="PSUM") as dps,
            ):
                with nc.named_scope("attn"):
                    for h in range(HPC):
                        for ic in range(IC):
                            isb = ic * ICW
                            p_sb = pbp.tile([128, JT, ICW], F16, tag="psb")
                            for jt in range(JT):
                                sp = aps.tile([128, ICW], F32, tag="sp", bufs=2)
                                for hf in range(2):
                                    nc.tensor.matmul(
                                        sp[:, hf * 512:hf * 512 + 512],
                                        kcat[h][:, jt * 128:jt * 128 + 128],
                                        qcat[h][:, isb + hf * 512:
                                                isb + hf * 512 + 512],
                                        start=True, stop=True)
                                nc.scalar.activation(
                                    out=p_sb[:, jt, :], in_=sp,
                                    func=AF.Exp, scale=SCALE)
                            if debug and h == 0 and ic == 0:
                                pcast = small.tile([128, ICW], F32, tag="dbgp", name="dbgp", bufs=1)
                                for jt2 in range(JT):
                                    nc.vector.tensor_copy(out=pcast, in_=p_sb[:, jt2, :])
                                    nc.sync.dma_start(out=dbg["p0"][:, jt2, :], in_=pcast)
                            for ib in range(ICW // 128):
                                ap2 = aps.tile([128, DA], F32, tag="av", bufs=2)
                                for jt in range(JT):
                                    nc.tensor.matmul(
                                        ap2, p_sb[:, jt, ib * 128:ib * 128 + 128],
                                        v_sb[:, jt, h, :],
                                        start=(jt == 0), stop=(jt == JT - 1))
                                r_t = small.tile([128, 1], F32, tag="r")
                                nc.vector.reciprocal(out=r_t, in_=ap2[:, D:DA])
                                av_t = small.tile([128, D], F16, tag="avsb")
                                nc.vector.tensor_scalar_mul(out=av_t,
                                                            in0=ap2[:, 0:D],
                                                            scalar1=r_t)
                                tp = aps.tile([64, 128], F16, tag="tp", bufs=1)
                                nc.tensor.transpose(tp, av_t, ident)
                                gi = isb + ib * 128
                                nc.vector.tensor_copy(
                                    out=avT[h // 2][64 * (h % 2):64 * (h % 2) + 64,
                                                    gi:gi + 128],
                                    in_=tp)
                        if h % 2 == 1:
                            pp = h // 2
                            if debug and pp == 0:
                                acast = small.tile([128, NCHW], F32, tag="dbga", name="dbga", bufs=1)
                                for nch2 in range(NCH):
                                    c2 = slice(nch2 * NCHW, nch2 * NCHW + NCHW)
                                    nc.vector.tensor_copy(out=acast, in_=avT[0][:, c2])
                                    nc.sync.dma_start(out=dbg["avT0"][:, c2], in_=acast)
                            nc.sync.dma_start(out=ag_in[pp], in_=avT[pp])
                            nc.gpsimd.collective_compute(
                                "AllGather", ALU.bypass, replica_groups=GROUPS4,
                                ins=[ag_in[pp].opt()], outs=[ag_out[pp].opt()])

                with nc.named_scope("dense"):
                    avfull = dnp.tile([128, KT, N], F16, tag="avfull")
                    avfull4 = avfull.rearrange("p (s t) n -> p s t n", t=2)
                    dense0 = dnp.tile([128, JT, DHC], F32, tag="d0")
                    # even half: pair-0 head columns, available right after AG0 —
                    # the scheduler overlaps this with attention on heads 2/3
                    nc.sync.dma_start(out=avfull4[:, :, 0, :],
                                      in_=ag_out[0].rearrange("s p n -> p s n"))
                    xres_v = xres.rearrange("(ib p) c -> p ib c", p=128)
                    for ib in range(JT):
                        xr = dscp.tile([128, DHC], F32, tag="xr")
                        nc.sync.dma_start(out=xr, in_=xres_v[:, ib, :])
                        pd = dps.tile([128, DHC], F32, tag="pd", bufs=1)
                        for s in range(4):
                            nc.tensor.matmul(pd, avfull4[:, s, 0,
                                                         ib * 128:ib * 128 + 128],
                                             wo_t[:, 2 * s, :],
                                             start=(s == 0), stop=(s == 3))
                        nc.vector.tensor_add(out=dense0[:, ib, :], in0=pd, in1=xr)
                    # odd half + stats, accumulated in place into dense0
                    nc.sync.dma_start(out=avfull4[:, :, 1, :],
                                      in_=ag_out[1].rearrange("s p n -> p s n"))
                    dense_t = dense0
                    stats = dnp.tile([128, JT, 2], F32, tag="stats")
                    for ib in range(JT):
                        pdt = aps.tile([128, ICW], F32, tag="sp", bufs=2, name=f"pdo{ib}")
                        pd = pdt[:, 0:DHC]
                        for s in range(4):
                            nc.tensor.matmul(pd, avfull4[:, s, 1,
                                                         ib * 128:ib * 128 + 128],
                                             wo_t[:, 2 * s + 1, :],
                                             start=(s == 0), stop=(s == 3))
                        dt_i = dense_t[:, ib, :]
                        nc.vector.tensor_add(out=dt_i, in0=pd, in1=dt_i)
                        if use_bo:
                            nc.vector.tensor_add(out=dt_i, in0=dt_i, in1=bo_rep)
                        nc.vector.reduce_sum(stats[:, ib, 0:1], dt_i, axis=AX.X)
                        sq = dscp.tile([128, DHC], F32, tag="sq")
                        nc.scalar.activation(out=sq, in_=dt_i, func=AF.Square,
                                             accum_out=stats[:, ib, 1:2])
                    if debug:
                        nc.sync.dma_start(out=dbg["avfull"], in_=avfull)
                        nc.sync.dma_start(out=dbg["stats"], in_=stats)
                        nc.sync.dma_start(out=dbg["dense0"], in_=dense_t)
                    ar_in = dram.tile([N, 2], F32, tag="arin")
                    ar_out = dram.tile([N, 2], F32, tag="arout")
                    nc.sync.dma_start(
                        out=ar_in.rearrange("(ib p) s -> p ib s", p=128), in_=stats)
                    nc.gpsimd.collective_compute(
                        "AllReduce", ALU.add, replica_groups=GROUPS4,
                        ins=[ar_in.opt()], outs=[ar_out.opt()])
                    stats2 = dnp.tile([128, JT, 2], F32, tag="stats2")
                    nc.sync.dma_start(
                        out=stats2, in_=ar_out.rearrange("(ib p) s -> p ib s", p=128))
                    inv_hid = 1.0 / HID
                    m_all = dnp.tile([128, JT], F32, tag="mall")
                    v_all = dnp.tile([128, JT], F32, tag="vall")
                    sq_all = dnp.tile([128, JT], F32, tag="sqall")
                    nc.vector.tensor_scalar_mul(out=m_all, in0=stats2[:, :, 0],
                                                scalar1=inv_hid)
                    nc.vector.tensor_mul(out=sq_all, in0=m_all, in1=m_all)
                    nc.vector.tensor_scalar_mul(out=v_all, in0=stats2[:, :, 1],
                                                scalar1=inv_hid)
                    nc.vector.tensor_sub(out=v_all, in0=v_all, in1=sq_all)
                    nc.scalar.activation(out=v_all, in_=v_all, func=AF.Sqrt,
                                         bias=eps_t)
                    nc.vector.reciprocal(out=v_all, in_=v_all)
                    for ib in range(JT):
                        o_t = dscp.tile([128, DHC], F32, tag="ot")
                        nc.vector.tensor_scalar(out=o_t, in0=dense_t[:, ib, :],
                                                scalar1=m_all[:, ib:ib + 1],
                                                scalar2=v_all[:, ib:ib + 1],
                                                op0=ALU.subtract, op1=ALU.mult)
                        if use_g:
                            nc.vector.tensor_mul(out=o_t, in0=o_t, in1=g_rep)
                        if use_b:
                            nc.vector.tensor_add(out=o_t, in0=o_t, in1=b_rep)
                        nc.sync.dma_start(
                            out=out.rearrange("(ib p) c -> p ib c", p=128)[:, ib, :],
                            in_=o_t)

    nc.compile()
    return nc


def _flags(inp):
    return (bool(np.any(inp["bq"]) or np.any(inp["bk"]) or np.any(inp["bpq"])
                 or np.any(inp["bpk"])),
            bool(np.any(inp["bo"])),
            bool(np.any(np.asarray(inp["ln_g"]) != 1.0)),
            bool(np.any(inp["ln_b"])))


def _make_in_maps(inp, flags):
    use_qk_bias, use_bo, use_g, use_b = flags
    x = np.asarray(inp["hidden_states"], np.float32)
    xT = [np.ascontiguousarray(x[b].T) for b in range(B)]
    posT = np.ascontiguousarray(np.asarray(inp["pos_emb"], np.float32).T)
    in_maps = []
    for c in range(NC):
        b, q = c // 4, c % 4
        hs = slice(DHC * q, DHC * q + DHC)
        im = {
            "xT": xT[b].astype(np.float16),
            "posT": posT.astype(np.float16),
            "wq": np.ascontiguousarray(np.asarray(inp["Wq"], np.float32)[:, hs]).astype(np.float16),
            "wk": np.ascontiguousarray(np.asarray(inp["Wk"], np.float32)[:, hs]).astype(np.float16),
            "wv": np.ascontiguousarray(np.asarray(inp["Wv"], np.float32)[:, hs]).astype(np.float16),
            "wpq": np.ascontiguousarray(np.asarray(inp["Wpq"], np.float32)[:, hs]).astype(np.float16),
            "wpk": np.ascontiguousarray(np.asarray(inp["Wpk"], np.float32)[:, hs]).astype(np.float16),
            "wo": np.ascontiguousarray(np.asarray(inp["Wo"], np.float32)[:, hs])
                    .astype(np.float16),
            "xres": np.ascontiguousarray(x[b][:, hs]),
            "bv": np.ascontiguousarray(np.asarray(inp["bv"], np.float32)[hs]),
        }
        if use_qk_bias:
            bq = np.asarray(inp["bq"], np.float32)
            bk = np.asarray(inp["bk"], np.float32)
            bpq = np.asarray(inp["bpq"], np.float32)
            bpk = np.asarray(inp["bpk"], np.float32)
            qb = np.zeros((128, HPC), np.float32)
            kb = np.zeros((128, HPC), np.float32)
            for hh in range(HPC):
                ds = slice(DHC * q + D * hh, DHC * q + D * hh + D)
                qb[0:64, hh] = bq[ds] + bpq[ds]
                qb[64:128, hh] = bq[ds]
                kb[0:64, hh] = bk[ds]
                kb[64:128, hh] = bpk[ds]
            im["qbias"], im["kbias"] = qb, kb
        if use_bo:
            im["bo"] = np.ascontiguousarray(np.asarray(inp["bo"], np.float32)[hs])
        if use_g:
            im["ln_g"] = np.ascontiguousarray(np.asarray(inp["ln_g"], np.float32)[hs])
        if use_b:
            im["ln_b"] = np.ascontiguousarray(np.asarray(inp["ln_b"], np.float32)[hs])
        in_maps.append(im)
    return in_maps


def _assemble(results):
    out = np.empty((B, N, HID), np.float32)
    for c in range(NC):
        b, q = c // 4, c % 4
        out[b, :, DHC * q:DHC * q + DHC] = results[c]["out"]
    return out


_STATE = {}


def kernel(hidden_states, Wq, bq, Wk, bk, Wv, bv, pos_emb, Wpq, bpq, Wpk, bpk,
           Wo, bo, ln_g, ln_b):
    inp = dict(hidden_states=hidden_states, Wq=Wq, bq=bq, Wk=Wk, bk=bk, Wv=Wv,
               bv=bv, pos_emb=pos_emb, Wpq=Wpq, bpq=bpq, Wpk=Wpk, bpk=bpk,
               Wo=Wo, bo=bo, ln_g=ln_g, ln_b=ln_b)
    flags = _flags(inp)
    if flags not in _STATE:
        _STATE[flags] = _build(flags)
    nc = _STATE[flags]
    res = run_bass_kernel_spmd(nc, _make_in_maps(inp, flags), list(range(NC)))
    return _assemble(res.results)
